# revision 8
# baseline (speedup 1.0000x reference)
"""Self-contained Trainium2 Bass kernel for nn_MultiHeadMPSRecurrence.

Reference computation (B=2, T=4096, D_MODEL=1024, D_HIDDEN=2048, K=4):
    ih    = causal_depthwise_conv(x @ W_ih + b, conv_w, conv_b)
    gate  = sigmoid(x @ W_gate + b)
    a     = sigmoid(x @ W_decay + b)
    z     = silu(x @ W_z + b)
    alpha = (1-gate)*a ; beta = gate*ih
    h     = scan(h_t = alpha_t*h_{t-1} + beta_t)
    out   = ((z * layernorm(h)) @ W_out + b) @ out_w + b

Sharding: 8 cores = 2 batches x 4 time-chunks of 1024 steps.  Each core runs
the full pipeline on its chunk; the sequential scan is chunk-linked through a
small AllGather of per-chunk (prod(alpha), last-local-state) vectors followed
by an on-device prefix combine, then each core re-runs its scan with the true
incoming state (native tensor_tensor_scan ISA op, fp32 internal state).

On-chip layout: channels on partitions (16 tiles of 128ch), time on free dim.
Matmuls run in bf16 with fp32 PSUM accumulation; elementwise/scan math in
fp32.  Host pre-fuses W_out @ out_w into one [2048,1024] matrix.
"""

import functools
from contextlib import ExitStack

import ml_dtypes
import numpy as np

import concourse.bass as bass
import concourse.tile as tile
from concourse import bacc, mybir
from concourse.bass_utils import run_bass_kernel_spmd

BF16 = mybir.dt.bfloat16
F32 = mybir.dt.float32
AF = mybir.ActivationFunctionType
OP = mybir.AluOpType
AX = mybir.AxisListType

B, T, D, H = 2, 4096, 1024, 2048
NCORES = 8
KCHUNKS = 4          # time chunks per batch
TC = T // KCHUNKS    # 1024 timesteps per core
HALO = 3             # conv taps reaching back in time (K-1)
TCX = TC + HALO      # 1027
NCT = H // 128       # 16 channel tiles
NDT = D // 128       # 8 d_model tiles
LN_EPS = 1e-5


def _build_program(has_out_bias: bool, has_ln_b: bool):
    nc = bacc.Bacc("TRN2", target_bir_lowering=False, debug=False,
                   enable_asserts=True, num_devices=NCORES)

    # ---------------- I/O ----------------
    xT = nc.dram_tensor("xT", [D, TCX], BF16, kind="ExternalInput").ap()
    w4 = nc.dram_tensor("w4", [D, 4 * H], BF16, kind="ExternalInput").ap()
    wcomb = nc.dram_tensor("wcomb", [H, D], BF16, kind="ExternalInput").ap()
    # per-channel vectors in [partition, ch_tile] layout
    bias_ih = nc.dram_tensor("bias_ih", [128, NCT], F32, kind="ExternalInput").ap()
    bias_halo = nc.dram_tensor("bias_halo", [128, NCT], F32, kind="ExternalInput").ap()
    bias_gneg = nc.dram_tensor("bias_gneg", [128, NCT], F32, kind="ExternalInput").ap()
    bias_dec = nc.dram_tensor("bias_dec", [128, NCT], F32, kind="ExternalInput").ap()
    bias_z = nc.dram_tensor("bias_z", [128, NCT], F32, kind="ExternalInput").ap()
    conv_b_d = nc.dram_tensor("conv_b", [128, NCT], F32, kind="ExternalInput").ap()
    ln_g_d = nc.dram_tensor("ln_g", [128, NCT], F32, kind="ExternalInput").ap()
    convw_d = nc.dram_tensor("convw", [128, NCT * 4], F32, kind="ExternalInput").ap()
    onehot_d = nc.dram_tensor("onehot", [128, NCORES], F32, kind="ExternalInput").ap()
    if has_out_bias:
        out_bias_d = nc.dram_tensor("out_bias", [128, D], F32, kind="ExternalInput").ap()
    if has_ln_b:
        ln_b_d = nc.dram_tensor("ln_b", [128, NCT], F32, kind="ExternalInput").ap()
    out = nc.dram_tensor("out", [TC, D], F32, kind="ExternalOutput").ap()

    with tile.TileContext(nc) as tc, ExitStack() as ctx:
        dram = ctx.enter_context(tc.tile_pool(name="dram", bufs=1, space="DRAM"))
        alpha_d = dram.tile([H, TC], BF16)
        beta_d = dram.tile([H, TC], BF16)
        cc_in = dram.tile([1, 2 * H], F32)
        cc_out = dram.tile([NCORES, 2 * H], F32, addr_space="Shared")
        rstd_d = dram.tile([1, TC], F32)

        # ---------------- constants / small resident ----------------
        consts = ctx.enter_context(tc.tile_pool(name="consts", bufs=1))
        bih_s = consts.tile([128, NCT], F32)
        nc.sync.dma_start(bih_s[:], bias_ih[:, :])
        bhalo_s = consts.tile([128, NCT], F32)
        nc.sync.dma_start(bhalo_s[:], bias_halo[:, :])
        bgn_s = consts.tile([128, NCT], F32)
        nc.sync.dma_start(bgn_s[:], bias_gneg[:, :])
        bdec_s = consts.tile([128, NCT], F32)
        nc.sync.dma_start(bdec_s[:], bias_dec[:, :])
        bz_s = consts.tile([128, NCT], F32)
        nc.sync.dma_start(bz_s[:], bias_z[:, :])
        cb_s = consts.tile([128, NCT], F32)
        nc.sync.dma_start(cb_s[:], conv_b_d[:, :])
        lng_s = consts.tile([128, NCT], F32)
        nc.sync.dma_start(lng_s[:], ln_g_d[:, :])
        cw_s = consts.tile([128, NCT * 4], F32)
        nc.sync.dma_start(cw_s[:], convw_d[:, :])
        oh_s = consts.tile([128, NCORES], F32)
        nc.sync.dma_start(oh_s[:], onehot_d[:, :])
        if has_out_bias:
            obias_s = consts.tile([128, D], F32)
            nc.sync.dma_start(obias_s[:], out_bias_d[:, :])
        if has_ln_b:
            lnb_s = consts.tile([128, NCT], F32)
            nc.sync.dma_start(lnb_s[:], ln_b_d[:, :])
        ones_stat = consts.tile([128, 1], BF16)
        nc.vector.memset(ones_stat[:], 1.0)
        ones_bc = consts.tile([1, 128], F32)
        nc.vector.memset(ones_bc[:], 1.0)
        ones_row = consts.tile([1, TC], F32)
        nc.vector.memset(ones_row[:], 1.0)

        # ---------------- big resident tiles ----------------
        res = ctx.enter_context(tc.tile_pool(name="res", bufs=1))
        z_s = res.tile([128, NCT * TC], BF16)
        h_s = res.tile([128, NCT * TC], BF16)
        wcomb_s = res.tile([128, NCT * D], BF16)
        nc.sync.dma_start(
            wcomb_s[:].rearrange("p (ct c) -> p ct c", c=D),
            wcomb.rearrange("(ct p) c -> p ct c", p=128),
        )
        if has_ln_b:
            qb_s = res.tile([128, NCT * TC], BF16)
        rows = ctx.enter_context(tc.tile_pool(name="rows", bufs=1))
        Pbuf = res.tile([128, NCT], F32)
        Lbuf = res.tile([128, NCT], F32)
        h_in = res.tile([128, NCT], F32)
        Pall = res.tile([128, NCORES * NCT], F32)
        Lall = res.tile([128, NCORES * NCT], F32)
        Hp = res.tile([128, NCORES * NCT], F32)
        rstd_col = res.tile([128, NDT], F32)

        # ================= phase 1: matmuls + gates + local scan =========
        with tc.tile_pool(name="p1xt", bufs=1) as xtpool:
            xT_s = xtpool.tile([128, NDT * TCX], BF16)
            nc.sync.dma_start(
                xT_s[:].rearrange("p (dt t) -> p dt t", t=TCX),
                xT.rearrange("(dt p) t -> p dt t", p=128),
            )

            with tc.tile_pool(name="p1w", bufs=2) as wpool, \
                 tc.tile_pool(name="p1ps", bufs=6, space="PSUM") as ppool, \
                 tc.tile_pool(name="p1f", bufs=2) as fpool, \
                 tc.tile_pool(name="p1b", bufs=2) as bpool:
                for ct in range(NCT):
                    # stream this channel-tile's weights for all 4 matrices
                    w_t = wpool.tile([128, 4 * NDT * 128], BF16)
                    for m in range(4):
                        nc.sync.dma_start(
                            w_t[:, m * NDT * 128:(m + 1) * NDT * 128]
                            .rearrange("p (dt c) -> p dt c", c=128),
                            w4[:, m * H + ct * 128: m * H + (ct + 1) * 128]
                            .rearrange("(dt p) c -> p dt c", p=128),
                        )

                    def mmgroup(m, lo, n):
                        ps = ppool.tile([128, 512], F32, tag="ps")
                        for dt in range(NDT):
                            nc.tensor.matmul(
                                ps[:, 0:n],
                                w_t[:, (m * NDT + dt) * 128:(m * NDT + dt + 1) * 128],
                                xT_s[:, dt * TCX + lo: dt * TCX + lo + n],
                                start=(dt == 0), stop=(dt == NDT - 1),
                            )
                        return ps

                    # m=0: ih over all 1027 halo columns
                    ih_pre = fpool.tile([128, TCX], F32, tag="ihpre")
                    for (lo, n, bias) in ((0, HALO, bhalo_s), (HALO, 512, bih_s),
                                          (HALO + 512, 512, bih_s)):
                        ps = mmgroup(0, lo, n)
                        nc.scalar.activation(ih_pre[:, lo:lo + n], ps[:, 0:n],
                                             AF.Identity,
                                             bias=bias[:, ct:ct + 1])
                    # m=1: gp = sigmoid(-(x@Wg + bg)) = 1-gate
                    gp = fpool.tile([128, TC], F32, tag="gp")
                    for half in range(2):
                        ps = mmgroup(1, HALO + half * 512, 512)
                        nc.scalar.activation(gp[:, half * 512:(half + 1) * 512],
                                             ps[:, :], AF.Sigmoid,
                                             bias=bgn_s[:, ct:ct + 1], scale=-1.0)
                    # m=2: a = sigmoid(x@Wd + bd)
                    a_t = fpool.tile([128, TC], F32, tag="a")
                    for half in range(2):
                        ps = mmgroup(2, HALO + half * 512, 512)
                        nc.scalar.activation(a_t[:, half * 512:(half + 1) * 512],
                                             ps[:, :], AF.Sigmoid,
                                             bias=bdec_s[:, ct:ct + 1])
                    # m=3: z = silu(x@Wz + bz), straight into resident bf16
                    for half in range(2):
                        ps = mmgroup(3, HALO + half * 512, 512)
                        nc.scalar.activation(
                            z_s[:, ct * TC + half * 512: ct * TC + (half + 1) * 512],
                            ps[:, :], AF.Silu, bias=bz_s[:, ct:ct + 1])

                    # causal depthwise conv (4 taps, halo in ih_pre)
                    ihc = fpool.tile([128, TC], F32, tag="ihc")
                    nc.scalar.activation(ihc[:], ih_pre[:, 3:3 + TC], AF.Identity,
                                         bias=cb_s[:, ct:ct + 1],
                                         scale=cw_s[:, ct * 4 + 3: ct * 4 + 4])
                    nc.vector.scalar_tensor_tensor(
                        ihc[:], ih_pre[:, 2:2 + TC], cw_s[:, ct * 4 + 2: ct * 4 + 3],
                        ihc[:], OP.mult, OP.add)
                    nc.vector.scalar_tensor_tensor(
                        ihc[:], ih_pre[:, 1:1 + TC], cw_s[:, ct * 4 + 1: ct * 4 + 2],
                        ihc[:], OP.mult, OP.add)
                    nc.vector.scalar_tensor_tensor(
                        ihc[:], ih_pre[:, 0:TC], cw_s[:, ct * 4: ct * 4 + 1],
                        ihc[:], OP.mult, OP.add)

                    # alpha = gp*a ; then reuse the a tile: gate = 1 - gp ;
                    # beta = gate*ihc
                    alpha_t = bpool.tile([128, TC], BF16, tag="alpha")
                    nc.vector.tensor_tensor(alpha_t[:], gp[:], a_t[:], OP.mult)
                    nc.scalar.activation(a_t[:], gp[:], AF.Copy,
                                         bias=1.0, scale=-1.0)
                    beta_t = bpool.tile([128, TC], BF16, tag="beta")
                    nc.gpsimd.tensor_tensor(beta_t[:], a_t[:], ihc[:], OP.mult)
                    nc.sync.dma_start(alpha_d[ct * 128:(ct + 1) * 128, :], alpha_t[:])
                    nc.sync.dma_start(beta_d[ct * 128:(ct + 1) * 128, :], beta_t[:])

                    # local scan (init 0) -> L ; running product -> P
                    h0_t = bpool.tile([128, TC], BF16, tag="h0")
                    nc.vector.tensor_tensor_scan(h0_t[:], alpha_t[:], beta_t[:],
                                                 0.0, OP.mult, OP.add)
                    nc.vector.tensor_copy(Lbuf[:, ct:ct + 1], h0_t[:, TC - 1:TC])
                    nc.vector.tensor_reduce(Pbuf[:, ct:ct + 1], alpha_t[:],
                                            AX.X, OP.mult)

        # ================= phase 2: boundary exchange =====================
        nc.sync.dma_start(
            cc_in[0:1, 0:H].rearrange("o (ct p) -> p (o ct)", p=128), Pbuf[:])
        nc.sync.dma_start(
            cc_in[0:1, H:2 * H].rearrange("o (ct p) -> p (o ct)", p=128), Lbuf[:])
        nc.gpsimd.collective_compute(
            "AllGather", OP.bypass, replica_groups=[list(range(NCORES))],
            ins=[cc_in.opt()], outs=[cc_out.opt()])
        for j in range(NCORES):
            nc.sync.dma_start(
                Pall[:, j * NCT:(j + 1) * NCT],
                cc_out[j:j + 1, 0:H].rearrange("o (ct p) -> p (o ct)", p=128))
            nc.sync.dma_start(
                Lall[:, j * NCT:(j + 1) * NCT],
                cc_out[j:j + 1, H:2 * H].rearrange("o (ct p) -> p (o ct)", p=128))

        # prefix combine over chain positions (both batches, uniform SPMD)
        def S(j):
            return slice(j * NCT, (j + 1) * NCT)
        for j in range(NCORES):
            if j % KCHUNKS == 0:
                nc.vector.tensor_copy(Hp[:, S(j)], Lall[:, S(j)])
            else:
                nc.vector.tensor_tensor(Hp[:, S(j)], Pall[:, S(j)],
                                        Hp[:, S(j - 1)], OP.mult)
                nc.vector.tensor_tensor(Hp[:, S(j)], Hp[:, S(j)],
                                        Lall[:, S(j)], OP.add)
        nc.vector.memset(h_in[:], 0.0)
        for j in range(NCORES):
            nc.vector.scalar_tensor_tensor(h_in[:], Hp[:, S(j)],
                                           oh_s[:, j:j + 1], h_in[:],
                                           OP.mult, OP.add)

        # ================= phase 3: true scan + LN stats ==================
        with tc.tile_pool(name="p3b", bufs=3) as p3pool, \
             tc.tile_pool(name="p3ps", bufs=1, space="PSUM") as spool:
            psum_sh = spool.tile([1, TC], F32)
            psum_sq = spool.tile([1, TC], F32)
            for ct in range(NCT):
                al3 = p3pool.tile([128, TC], BF16, tag="al3")
                nc.sync.dma_start(al3[:], alpha_d[ct * 128:(ct + 1) * 128, :])
                be3 = p3pool.tile([128, TC], BF16, tag="be3")
                nc.sync.dma_start(be3[:], beta_d[ct * 128:(ct + 1) * 128, :])
                hsl = h_s[:, ct * TC:(ct + 1) * TC]
                nc.vector.tensor_tensor_scan(hsl, al3[:], be3[:],
                                             h_in[:, ct:ct + 1], OP.mult, OP.add)
                h2 = p3pool.tile([128, TC], BF16, tag="h2")
                nc.scalar.activation(h2[:], hsl, AF.Square)
                for half in range(2):
                    nc.tensor.matmul(
                        psum_sh[0:1, half * 512:(half + 1) * 512],
                        ones_stat[:, 0:1],
                        h_s[:, ct * TC + half * 512: ct * TC + (half + 1) * 512],
                        start=(ct == 0), stop=(ct == NCT - 1))
                    nc.tensor.matmul(
                        psum_sq[0:1, half * 512:(half + 1) * 512],
                        ones_stat[:, 0:1],
                        h2[:, half * 512:(half + 1) * 512],
                        start=(ct == 0), stop=(ct == NCT - 1))

            # ============= phase 4: LN params + apply =====================
            mu = rows.tile([1, TC], F32)
            nc.scalar.activation(mu[:], psum_sh[:], AF.Copy, scale=1.0 / H)
            msq = rows.tile([1, TC], F32)
            nc.scalar.activation(msq[:], psum_sq[:], AF.Copy, scale=1.0 / H)
        mu2 = rows.tile([1, TC], F32)
        nc.vector.tensor_tensor(mu2[:], mu[:], mu[:], OP.mult)
        var_t = rows.tile([1, TC], F32)
        nc.vector.tensor_tensor(var_t[:], msq[:], mu2[:], OP.subtract)
        eps_t = rows.tile([1, 1], F32)
        nc.vector.memset(eps_t[:], LN_EPS)
        sd = rows.tile([1, TC], F32)
        nc.scalar.activation(sd[:], var_t[:], AF.Sqrt, bias=eps_t[:])
        rstd_t = rows.tile([1, TC], F32)
        nc.vector.reciprocal(rstd_t[:], sd[:])
        nc.sync.dma_start(rstd_d[0:1, :], rstd_t[:])
        nc.sync.dma_start(
            rstd_col[:], rstd_d[0:1, :].rearrange("o (tt p) -> p (o tt)", p=128))

        with tc.tile_pool(name="p4ps", bufs=1, space="PSUM") as bpool4, \
             tc.tile_pool(name="p4f", bufs=2) as p4pool:
            mu_b = bpool4.tile([128, TC], F32)
            for half in range(2):
                nc.tensor.matmul(mu_b[:, half * 512:(half + 1) * 512],
                                 ones_bc[0:1, :],
                                 mu[0:1, half * 512:(half + 1) * 512])
            for ct in range(NCT):
                sl = slice(ct * TC, (ct + 1) * TC)
                t1 = p4pool.tile([128, TC], F32, tag="t1")
                nc.vector.tensor_tensor(t1[:], h_s[:, sl], mu_b[:], OP.subtract)
                nc.vector.scalar_tensor_tensor(z_s[:, sl], t1[:],
                                               lng_s[:, ct:ct + 1], z_s[:, sl],
                                               OP.mult, OP.mult)
                if has_ln_b:
                    nc.vector.tensor_scalar(qb_s[:, sl], z_s[:, sl],
                                            lnb_s[:, ct:ct + 1], None, OP.mult)

            # ============= phase 5: output matmuls ========================
            with tc.tile_pool(name="p5ps", bufs=3, space="PSUM") as ypool, \
                 tc.tile_pool(name="p5o", bufs=3) as opool:
                for tt in range(NDT):
                    for cg in range(2):
                        y_ps = ypool.tile([128, 512], F32, tag="yps")
                        for ct in range(NCT):
                            nc.tensor.matmul(
                                y_ps[:],
                                z_s[:, ct * TC + tt * 128: ct * TC + (tt + 1) * 128],
                                wcomb_s[:, ct * D + cg * 512: ct * D + (cg + 1) * 512],
                                start=(ct == 0), stop=(ct == NCT - 1))
                        out_sb = opool.tile([128, 512], F32, tag="osb")
                        nc.scalar.activation(out_sb[:], y_ps[:], AF.Copy,
                                             scale=rstd_col[:, tt:tt + 1])
                        if has_ln_b:
                            y2_ps = ypool.tile([128, 512], F32, tag="y2ps")
                            for ct in range(NCT):
                                nc.tensor.matmul(
                                    y2_ps[:],
                                    qb_s[:, ct * TC + tt * 128: ct * TC + (tt + 1) * 128],
                                    wcomb_s[:, ct * D + cg * 512: ct * D + (cg + 1) * 512],
                                    start=(ct == 0), stop=(ct == NCT - 1))
                            nc.vector.tensor_tensor(out_sb[:], out_sb[:],
                                                    y2_ps[:], OP.add)
                        if has_out_bias:
                            nc.vector.tensor_tensor(
                                out_sb[:], out_sb[:],
                                obias_s[:, cg * 512:(cg + 1) * 512], OP.add)
                        nc.sync.dma_start(
                            out[tt * 128:(tt + 1) * 128,
                                cg * 512:(cg + 1) * 512], out_sb[:])

    nc.compile()
    return nc


@functools.lru_cache(maxsize=4)
def _program(flags):
    return _build_program(*flags)


def _v2m(v):
    return np.ascontiguousarray(np.asarray(v, np.float32).reshape(NCT, 128).T)


def kernel(x, W_ih_w, W_ih_b, W_gate_w, W_gate_b, W_decay_w, W_decay_b,
           W_z_w, W_z_b, conv_w, conv_b, ln_g, ln_b, W_out_w, W_out_b,
           out_w, out_b):
    f32 = np.float32
    bf = ml_dtypes.bfloat16
    x = np.asarray(x, f32)

    out_bias_eff = (np.asarray(W_out_b, f32) @ np.asarray(out_w, f32)
                    + np.asarray(out_b, f32))
    has_ob = bool(np.any(out_bias_eff != 0.0))
    has_lb = bool(np.any(np.asarray(ln_b) != 0.0))
    nc = _program((has_ob, has_lb))

    w4 = np.concatenate([np.asarray(W_ih_w, f32), np.asarray(W_gate_w, f32),
                         np.asarray(W_decay_w, f32), np.asarray(W_z_w, f32)],
                        axis=1).astype(bf)
    wcomb = (np.asarray(W_out_w, f32) @ np.asarray(out_w, f32)).astype(bf)
    convw_m = np.ascontiguousarray(
        np.asarray(conv_w, f32).reshape(NCT, 128, 4).transpose(1, 0, 2)
        .reshape(128, NCT * 4))
    bias_ih_m = _v2m(W_ih_b)
    shared = dict(
        w4=w4, wcomb=wcomb, bias_ih=bias_ih_m, bias_gneg=_v2m(-np.asarray(W_gate_b, f32)),
        bias_dec=_v2m(W_decay_b), bias_z=_v2m(W_z_b), conv_b=_v2m(conv_b),
        ln_g=_v2m(ln_g), convw=convw_m)
    if has_ob:
        shared["out_bias"] = np.ascontiguousarray(
            np.tile(out_bias_eff[None, :], (128, 1)).astype(f32))
    if has_lb:
        shared["ln_b"] = _v2m(ln_b)

    zero_halo = np.zeros((128, NCT), f32)
    in_maps = []
    for c in range(NCORES):
        b, k = divmod(c, KCHUNKS)
        t0 = k * TC
        if k == 0:
            xc = np.vstack([np.zeros((HALO, D), f32), x[b, :TC]])
        else:
            xc = x[b, t0 - HALO: t0 + TC]
        xTc = np.ascontiguousarray(xc.T).astype(bf)
        oh = np.zeros(NCORES, f32)
        if k > 0:
            oh[b * KCHUNKS + k - 1] = 1.0
        in_maps.append({
            **shared,
            "xT": xTc,
            "bias_halo": bias_ih_m if k > 0 else zero_halo,
            "onehot": np.ascontiguousarray(np.tile(oh[None, :], (128, 1))),
        })

    res = run_bass_kernel_spmd(nc, in_maps, core_ids=list(range(NCORES)))

    outf = np.empty((B, T, D), f32)
    for c in range(NCORES):
        b, k = divmod(c, KCHUNKS)
        outf[b, k * TC:(k + 1) * TC, :] = res.results[c]["out"]
    return outf


# revision 21
# speedup vs baseline: 1.0200x; 1.0200x over previous
"""Self-contained Trainium2 Bass kernel for nn_MultiHeadMPSRecurrence.

Reference computation (B=2, T=4096, D_MODEL=1024, D_HIDDEN=2048, K=4):
    ih    = causal_depthwise_conv(x @ W_ih + b, conv_w, conv_b)
    gate  = sigmoid(x @ W_gate + b)
    a     = sigmoid(x @ W_decay + b)
    z     = silu(x @ W_z + b)
    alpha = (1-gate)*a ; beta = gate*ih
    h     = scan(h_t = alpha_t*h_{t-1} + beta_t)
    out   = ((z * layernorm(h)) @ W_out + b) @ out_w + b

Sharding: 8 cores = 2 batches x 4 time-chunks of 1024 steps.  Each core runs
the full pipeline on its chunk; the sequential scan is chunk-linked through
AllGathers of per-chunk (prod(alpha), last-local-state) vectors followed by an
on-device prefix combine, then each core re-runs its scan with the true
incoming state (native tensor_tensor_scan ISA op, fp32 internal state).  The
exchange is split into two collectives so most of the post-exchange work
overlaps the tail of the main matmul phase.

On-chip layout: channels on partitions (16 tiles of 128ch), time on free dim.
Matmuls run in bf16 with fp32 PSUM accumulation; elementwise/scan math in
fp32.  Host pre-fuses W_out @ out_w into one [2048,1024] matrix.
"""

import functools
from contextlib import ExitStack

import ml_dtypes
import numpy as np

import concourse.bass as bass
import concourse.tile as tile
from concourse import bacc, mybir
from concourse.bass_utils import run_bass_kernel_spmd

BF16 = mybir.dt.bfloat16
F32 = mybir.dt.float32
AF = mybir.ActivationFunctionType
OP = mybir.AluOpType
AX = mybir.AxisListType

B, T, D, H = 2, 4096, 1024, 2048
NCORES = 8
KCHUNKS = 4          # time chunks per batch
TC = T // KCHUNKS    # 1024 timesteps per core
HALO = 3             # conv taps reaching back in time (K-1)
TCX = TC + HALO      # 1027
NCT = H // 128       # 16 channel tiles
NDT = D // 128       # 8 d_model tiles
LN_EPS = 1e-5
SPLIT = 12           # channel tiles covered by the first boundary exchange


def _build_program(has_out_bias: bool, has_ln_b: bool, sim_no_cc: bool = False):
    nc = bacc.Bacc("TRN2", target_bir_lowering=False, debug=False,
                   enable_asserts=True, num_devices=NCORES)

    # ---------------- I/O ----------------
    xT = nc.dram_tensor("xT", [D, TCX], BF16, kind="ExternalInput").ap()
    w4 = nc.dram_tensor("w4", [D, 4 * H], BF16, kind="ExternalInput").ap()
    wcomb = nc.dram_tensor("wcomb", [H, D], BF16, kind="ExternalInput").ap()
    # per-channel vectors in [partition, ch_tile] layout
    def cvec(name, n=NCT):
        return nc.dram_tensor(name, [128, n], F32, kind="ExternalInput").ap()
    bias_ih = cvec("bias_ih")
    bias_halo = cvec("bias_halo")
    bias_gneg = cvec("bias_gneg")
    bias_dec = cvec("bias_dec")
    bias_z = cvec("bias_z")
    conv_b_d = cvec("conv_b")
    ln_g_d = cvec("ln_g")
    convw_d = cvec("convw", NCT * 4)
    onehot_d = cvec("onehot", NCORES)
    if has_out_bias:
        out_bias_d = cvec("out_bias", D)
    if has_ln_b:
        ln_b_d = cvec("ln_b")
    out = nc.dram_tensor("out", [TC, D], F32, kind="ExternalOutput").ap()

    with tile.TileContext(nc) as tc, ExitStack() as ctx:
        dram = ctx.enter_context(tc.tile_pool(name="dram", bufs=1, space="DRAM"))
        alpha_d = dram.tile([H, TC], BF16)
        beta_d = dram.tile([H, TC], BF16)
        n1, n2 = SPLIT, NCT - SPLIT
        cc1_in = dram.tile([1, 2 * n1 * 128], F32)
        cc1_out = dram.tile([NCORES, 2 * n1 * 128], F32, addr_space="Shared")
        cc2_in = dram.tile([1, 2 * n2 * 128], F32)
        cc2_out = dram.tile([NCORES, 2 * n2 * 128], F32, addr_space="Shared")
        rstd_d = dram.tile([2, TC], F32)

        # ---------------- constants / small resident ----------------
        consts = ctx.enter_context(tc.tile_pool(name="consts", bufs=1))

        def load_const(name, ap_in, n):
            t = consts.tile([128, n], F32, tag=name)
            nc.sync.dma_start(t[:], ap_in[:, :])
            return t
        bih_s = load_const("bih", bias_ih, NCT)
        bhalo_s = load_const("bhalo", bias_halo, NCT)
        bgn_s = load_const("bgn", bias_gneg, NCT)
        bdec_s = load_const("bdec", bias_dec, NCT)
        bz_s = load_const("bz", bias_z, NCT)
        cb_s = load_const("cb", conv_b_d, NCT)
        lng_s = load_const("lng", ln_g_d, NCT)
        cw_s = load_const("cw", convw_d, NCT * 4)
        oh_s = load_const("oh", onehot_d, NCORES)
        if has_out_bias:
            obias_s = load_const("obias", out_bias_d, D)
        if has_ln_b:
            lnb_s = load_const("lnb", ln_b_d, NCT)
        ones_stat = consts.tile([128, 1], BF16)
        nc.vector.memset(ones_stat[:], 1.0)
        ones_bc = consts.tile([1, 128], F32)
        nc.vector.memset(ones_bc[:], 1.0)

        # ---------------- big resident tiles ----------------
        res = ctx.enter_context(tc.tile_pool(name="res", bufs=1))
        z_s = res.tile([128, NCT * TC], BF16)
        h_s = res.tile([128, NCT * TC], BF16)
        wcomb_s = res.tile([128, NCT * D], BF16)
        if has_ln_b:
            qb_s = res.tile([128, NCT * TC], BF16)
        rows = ctx.enter_context(tc.tile_pool(name="rows", bufs=1))
        Pbuf = res.tile([128, NCT], F32)
        Lbuf = res.tile([128, NCT], F32)
        h_in = res.tile([128, NCT], F32)
        Pall = res.tile([128, NCORES * NCT], F32)
        Lall = res.tile([128, NCORES * NCT], F32)
        Hp = res.tile([128, NCORES * NCT], F32)
        rstd_col = res.tile([128, NDT], F32)
        cumA_s = res.tile([128, (NCT - SPLIT) * TC], BF16)
        mu_sb = rows.tile([128, TC], F32)
        mu_row = rows.tile([1, TC], F32)
        msq_row = rows.tile([1, TC], F32)
        mu = mu_row[:, :]
        msq = msq_row[:, :]
        cpack = rows.tile([128, 4 * NDT + 1], F32)
        mu_col = cpack[:, 0:NDT]
        msq_col = cpack[:, NDT:2 * NDT]
        var_col = cpack[:, 2 * NDT:3 * NDT]
        sd_col = cpack[:, 3 * NDT:4 * NDT]
        eps_t = cpack[:, 4 * NDT:4 * NDT + 1]

        def comm_half(lo, hi, cci, cco):
            """AllGather (P,L) for channel tiles [lo,hi); prefix-combine and
            select this core's incoming state into h_in[:, lo:hi]."""
            n = hi - lo
            nc.sync.dma_start(
                cci[0:1, 0:n * 128].rearrange("o (ct p) -> p (o ct)", p=128),
                Pbuf[:, lo:hi])
            nc.sync.dma_start(
                cci[0:1, n * 128:2 * n * 128]
                .rearrange("o (ct p) -> p (o ct)", p=128),
                Lbuf[:, lo:hi])
            if sim_no_cc:
                # TimelineSim can't model collectives; local DMA stand-in.
                nc.sync.dma_start(cco[0:1, :], cci[:, :])
            else:
                nc.gpsimd.collective_compute(
                    "AllGather", OP.bypass,
                    replica_groups=[list(range(NCORES))],
                    ins=[cci.opt()], outs=[cco.opt()])
            for j in range(NCORES):
                nc.sync.dma_start(
                    Pall[:, j * NCT + lo: j * NCT + hi],
                    cco[j:j + 1, 0:n * 128]
                    .rearrange("o (ct p) -> p (o ct)", p=128))
                nc.sync.dma_start(
                    Lall[:, j * NCT + lo: j * NCT + hi],
                    cco[j:j + 1, n * 128:2 * n * 128]
                    .rearrange("o (ct p) -> p (o ct)", p=128))
            for j in range(NCORES):
                sj = slice(j * NCT + lo, j * NCT + hi)
                sjm = slice((j - 1) * NCT + lo, (j - 1) * NCT + hi)
                if j % KCHUNKS == 0:
                    nc.vector.tensor_copy(Hp[:, sj], Lall[:, sj])
                else:
                    nc.vector.tensor_tensor(Hp[:, sj], Pall[:, sj],
                                            Hp[:, sjm], OP.mult)
                    nc.vector.tensor_tensor(Hp[:, sj], Hp[:, sj],
                                            Lall[:, sj], OP.add)
            nc.vector.memset(h_in[:, lo:hi], 0.0)
            for j in range(NCORES):
                sj = slice(j * NCT + lo, j * NCT + hi)
                nc.vector.scalar_tensor_tensor(
                    h_in[:, lo:hi], Hp[:, sj], oh_s[:, j:j + 1],
                    h_in[:, lo:hi], OP.mult, OP.add)

        # distribute first-half true-scans over the tail of phase 1
        p3_sched = {SPLIT + i: [] for i in range(NCT - SPLIT)}
        for i in range(SPLIT):
            p3_sched[SPLIT + min(i * (NCT - SPLIT) // SPLIT,
                                 NCT - SPLIT - 1)].append(i)

        with tc.tile_pool(name="p3ps", bufs=1, space="PSUM") as spool, \
             tc.tile_pool(name="p3b", bufs=2) as p3pool:
            psum_sh = spool.tile([1, TC], F32)
            psum_sq = spool.tile([1, TC], F32)

            def phase3_ct(ct):
                al3 = p3pool.tile([128, TC], BF16, tag="al3")
                nc.sync.dma_start(al3[:], alpha_d[ct * 128:(ct + 1) * 128, :])
                be3 = p3pool.tile([128, TC], BF16, tag="be3")
                nc.sync.dma_start(be3[:], beta_d[ct * 128:(ct + 1) * 128, :])
                hsl = h_s[:, ct * TC:(ct + 1) * TC]
                nc.vector.tensor_tensor_scan(hsl, al3[:], be3[:],
                                             h_in[:, ct:ct + 1],
                                             OP.mult, OP.add)
                for half in range(2):
                    nc.tensor.matmul(
                        psum_sh[0:1, half * 512:(half + 1) * 512],
                        ones_stat[:, 0:1],
                        h_s[:, ct * TC + half * 512: ct * TC + (half + 1) * 512],
                        start=(ct == 0), stop=(ct == NCT - 1))
                h2 = p3pool.tile([128, TC], BF16, tag="h2")
                nc.gpsimd.tensor_tensor(h2[:], hsl, hsl, OP.mult)
                for half in range(2):
                    nc.tensor.matmul(
                        psum_sq[0:1, half * 512:(half + 1) * 512],
                        ones_stat[:, 0:1],
                        h2[:, half * 512:(half + 1) * 512],
                        start=(ct == 0), stop=(ct == NCT - 1))

            # ============ phase 1: matmuls + gates + local scans ==========
            with tc.tile_pool(name="p1xt", bufs=1) as xtpool, \
                 tc.tile_pool(name="p1w", bufs=2) as wpool, \
                 tc.tile_pool(name="p1ps", bufs=4, space="PSUM") as ppool, \
                 tc.tile_pool(name="p1f", bufs=2) as fpool, \
                 tc.tile_pool(name="p1b", bufs=2) as bpool:
                xT_s = xtpool.tile([128, NDT * TCX], BF16)

                def load_w(ct):
                    w_t = wpool.tile([128, 4 * NDT * 128], BF16, tag="w_t")
                    for m in range(4):
                        nc.sync.dma_start(
                            w_t[:, m * NDT * 128:(m + 1) * NDT * 128]
                            .rearrange("p (dt c) -> p dt c", c=128),
                            w4[:, m * H + ct * 128: m * H + (ct + 1) * 128]
                            .rearrange("(dt p) c -> p dt c", p=128),
                        )
                    return w_t

                # first weight tile ahead of the bulk xT load so PE can start
                w_next = load_w(0)
                for dt in range(NDT):
                    nc.sync.dma_start(
                        xT_s[:, dt * TCX:(dt + 1) * TCX],
                        xT[dt * 128:(dt + 1) * 128, :].rearrange(
                            "(o p) t -> p (o t)", p=128))

                for ct in range(NCT):
                    w_t = w_next
                    if ct + 1 < NCT:
                        w_next = load_w(ct + 1)

                    def mmgroup(m, lo, n):
                        ps = ppool.tile([128, 512], F32, tag="ps")
                        for dt in range(NDT):
                            nc.tensor.matmul(
                                ps[:, 0:n],
                                w_t[:, (m * NDT + dt) * 128:
                                    (m * NDT + dt + 1) * 128],
                                xT_s[:, dt * TCX + lo: dt * TCX + lo + n],
                                start=(dt == 0), stop=(dt == NDT - 1),
                            )
                        return ps

                    # m=0: ih over all 1027 halo columns
                    ih_pre = fpool.tile([128, TCX], F32, tag="ihpre")
                    for (lo, n, bias) in ((0, HALO, bhalo_s),
                                          (HALO, 512, bih_s),
                                          (HALO + 512, 512, bih_s)):
                        ps = mmgroup(0, lo, n)
                        nc.scalar.activation(ih_pre[:, lo:lo + n], ps[:, 0:n],
                                             AF.Identity,
                                             bias=bias[:, ct:ct + 1])
                    # m=1: gp = sigmoid(-(x@Wg + bg)) = 1-gate
                    gp = fpool.tile([128, TC], F32, tag="gp")
                    for half in range(2):
                        ps = mmgroup(1, HALO + half * 512, 512)
                        nc.scalar.activation(
                            gp[:, half * 512:(half + 1) * 512], ps[:, :],
                            AF.Sigmoid, bias=bgn_s[:, ct:ct + 1], scale=-1.0)
                    # m=2: a = sigmoid(x@Wd + bd)
                    a_t = fpool.tile([128, TC], F32, tag="a")
                    for half in range(2):
                        ps = mmgroup(2, HALO + half * 512, 512)
                        nc.scalar.activation(
                            a_t[:, half * 512:(half + 1) * 512], ps[:, :],
                            AF.Sigmoid, bias=bdec_s[:, ct:ct + 1])
                    # m=3: z = silu(x@Wz + bz), straight into resident bf16
                    for half in range(2):
                        ps = mmgroup(3, HALO + half * 512, 512)
                        nc.scalar.activation(
                            z_s[:, ct * TC + half * 512:
                                ct * TC + (half + 1) * 512],
                            ps[:, :], AF.Silu, bias=bz_s[:, ct:ct + 1])

                    # causal depthwise conv (4 taps, halo in ih_pre)
                    ihc = fpool.tile([128, TC], F32, tag="ihc")
                    nc.scalar.activation(ihc[:], ih_pre[:, 3:3 + TC],
                                         AF.Identity,
                                         bias=cb_s[:, ct:ct + 1],
                                         scale=cw_s[:, ct * 4 + 3: ct * 4 + 4])
                    for j in (2, 1, 0):
                        nc.vector.scalar_tensor_tensor(
                            ihc[:], ih_pre[:, j:j + TC],
                            cw_s[:, ct * 4 + j: ct * 4 + j + 1],
                            ihc[:], OP.mult, OP.add)

                    # alpha = gp*a ; reuse a tile: gate = 1-gp ; beta = gate*ihc
                    alpha_t = bpool.tile([128, TC], BF16, tag="alpha")
                    nc.vector.tensor_tensor(alpha_t[:], gp[:], a_t[:], OP.mult)
                    nc.scalar.activation(a_t[:], gp[:], AF.Copy,
                                         bias=1.0, scale=-1.0)
                    beta_t = bpool.tile([128, TC], BF16, tag="beta")
                    nc.gpsimd.tensor_tensor(beta_t[:], a_t[:], ihc[:], OP.mult)
                    if ct < SPLIT:
                        nc.sync.dma_start(alpha_d[ct * 128:(ct + 1) * 128, :],
                                          alpha_t[:])
                        nc.sync.dma_start(beta_d[ct * 128:(ct + 1) * 128, :],
                                          beta_t[:])

                    # local scan (init 0) -> L ; running product -> P
                    if ct < SPLIT:
                        h0_t = bpool.tile([128, TC], BF16, tag="h0")
                        nc.vector.tensor_tensor_scan(h0_t[:], alpha_t[:],
                                                     beta_t[:], 0.0,
                                                     OP.mult, OP.add)
                        nc.vector.tensor_copy(Lbuf[:, ct:ct + 1],
                                              h0_t[:, TC - 1:TC])
                        nc.vector.tensor_reduce(Pbuf[:, ct:ct + 1], alpha_t[:],
                                                AX.X, OP.mult)
                    else:
                        # tail group: local scan lands in h_s; cumulative
                        # product of alpha kept for the post-exchange fixup
                        # h = h0 + cumA * h_in (no re-scan needed).
                        hsl = h_s[:, ct * TC:(ct + 1) * TC]
                        nc.vector.tensor_tensor_scan(hsl, alpha_t[:],
                                                     beta_t[:], 0.0,
                                                     OP.mult, OP.add)
                        casl = cumA_s[:, (ct - SPLIT) * TC:
                                      (ct - SPLIT + 1) * TC]
                        nc.vector.tensor_tensor_scan(casl, alpha_t[:],
                                                     alpha_t[:], 1.0,
                                                     OP.mult, OP.bypass)
                        nc.vector.tensor_copy(Lbuf[:, ct:ct + 1],
                                              hsl[:, TC - 1:TC])
                        nc.vector.tensor_copy(Pbuf[:, ct:ct + 1],
                                              casl[:, TC - 1:TC])

                    if ct == SPLIT - 1:
                        comm_half(0, SPLIT, cc1_in, cc1_out)
                    for c3 in p3_sched.get(ct, []):
                        phase3_ct(c3)

            # ============ second boundary exchange + fixups ===============
            comm_half(SPLIT, NCT, cc2_in, cc2_out)
            for ct in range(SPLIT, NCT):
                hsl = h_s[:, ct * TC:(ct + 1) * TC]
                casl = cumA_s[:, (ct - SPLIT) * TC:(ct - SPLIT + 1) * TC]
                nc.vector.scalar_tensor_tensor(
                    hsl, casl, h_in[:, ct:ct + 1], hsl, OP.mult, OP.add)
                for half in range(2):
                    nc.tensor.matmul(
                        psum_sh[0:1, half * 512:(half + 1) * 512],
                        ones_stat[:, 0:1],
                        h_s[:, ct * TC + half * 512: ct * TC + (half + 1) * 512],
                        start=(ct == 0), stop=(ct == NCT - 1))
                h2f = p3pool.tile([128, TC], BF16, tag="h2")
                nc.gpsimd.tensor_tensor(h2f[:], hsl, hsl, OP.mult)
                for half in range(2):
                    nc.tensor.matmul(
                        psum_sq[0:1, half * 512:(half + 1) * 512],
                        ones_stat[:, 0:1],
                        h2f[:, half * 512:(half + 1) * 512],
                        start=(ct == 0), stop=(ct == NCT - 1))

            # ============ LN parameters ===================================
            nc.scalar.activation(mu, psum_sh[:], AF.Copy, scale=1.0 / H)
            nc.scalar.activation(msq, psum_sq[:], AF.Copy, scale=1.0 / H)

        nc.sync.dma_start(rstd_d[0:1, :], mu)
        nc.sync.dma_start(rstd_d[1:2, :], msq)
        nc.sync.dma_start(
            cpack[:, 0:2 * NDT].rearrange("p (s tt) -> p s tt", s=2),
            rstd_d[0:2, :].rearrange("s (tt p) -> p s tt", p=128))
        nc.vector.memset(eps_t, LN_EPS)
        nc.vector.scalar_tensor_tensor(var_col, mu_col, -1.0, mu_col,
                                       OP.mult, OP.mult)
        nc.vector.tensor_tensor(var_col, msq_col, var_col, OP.add)
        nc.scalar.activation(sd_col, var_col, AF.Sqrt, bias=eps_t)
        nc.vector.reciprocal(rstd_col[:], sd_col)

        # broadcast mu across partitions, into SBUF
        with tc.tile_pool(name="p4ps", bufs=1, space="PSUM") as bpool4:
            mu_b = bpool4.tile([128, TC], F32)
            for half in range(2):
                nc.tensor.matmul(mu_b[:, half * 512:(half + 1) * 512],
                                 ones_bc[0:1, :],
                                 mu[:, half * 512:(half + 1) * 512])
            nc.scalar.activation(mu_sb[:], mu_b[:], AF.Copy)

        # load the fused output weight (deferred: keeps startup DMA free)
        nc.sync.dma_start(
            wcomb_s[:].rearrange("p (ct c) -> p ct c", c=D),
            wcomb.rearrange("(ct p) c -> p ct c", p=128),
        )

        # ===== phase 4+5 interleaved: LN apply feeding output matmuls =====
        # cg=0 accumulates per-ct as apply results land; cg=1 runs dense.
        with tc.tile_pool(name="p45f", bufs=4) as p4pool, \
             tc.tile_pool(name="p5ps", bufs=8, space="PSUM") as ypool, \
             tc.tile_pool(name="p5o", bufs=4) as opool:

            def evict_tt(tt, cg, y_ps):
                out_sb = opool.tile([128, 512], F32, tag="osb")
                nc.scalar.activation(out_sb[:], y_ps[:], AF.Copy,
                                     scale=rstd_col[:, tt:tt + 1])
                if has_ln_b:
                    y2 = ypool.tile([128, 512], F32, tag="y")
                    for ct in range(NCT):
                        nc.tensor.matmul(
                            y2[:],
                            qb_s[:, ct * TC + tt * 128: ct * TC + (tt + 1) * 128],
                            wcomb_s[:, ct * D + cg * 512:
                                    ct * D + (cg + 1) * 512],
                            start=(ct == 0), stop=(ct == NCT - 1))
                    nc.vector.tensor_tensor(out_sb[:], out_sb[:], y2[:], OP.add)
                if has_out_bias:
                    nc.vector.tensor_tensor(
                        out_sb[:], out_sb[:],
                        obias_s[:, cg * 512:(cg + 1) * 512], OP.add)
                nc.sync.dma_start(
                    out[tt * 128:(tt + 1) * 128, cg * 512:(cg + 1) * 512],
                    out_sb[:])

            ytiles = []
            for _tt in range(NDT):
                ybank = ypool.tile([128, 512], F32, tag="y")
                ytiles.append(ybank)
            for ct in range(NCT):
                sl = slice(ct * TC, (ct + 1) * TC)
                t1 = p4pool.tile([128, TC], F32, tag="t1")
                eng = nc.gpsimd if ct % 3 == 2 else nc.vector
                eng.tensor_tensor(t1[:], h_s[:, sl], mu_sb[:], OP.subtract)
                nc.vector.scalar_tensor_tensor(z_s[:, sl], t1[:],
                                               lng_s[:, ct:ct + 1], z_s[:, sl],
                                               OP.mult, OP.mult)
                if has_ln_b:
                    nc.vector.tensor_scalar(qb_s[:, sl], z_s[:, sl],
                                            lnb_s[:, ct:ct + 1], None, OP.mult)
                for tt in range(NDT):
                    nc.tensor.matmul(
                        ytiles[tt][:],
                        z_s[:, ct * TC + tt * 128: ct * TC + (tt + 1) * 128],
                        wcomb_s[:, ct * D: ct * D + 512],
                        start=(ct == 0), stop=(ct == NCT - 1))
            for tt in range(NDT):
                evict_tt(tt, 0, ytiles[tt])

            for tt in range(NDT):
                y_ps = ypool.tile([128, 512], F32, tag="y")
                for ct in range(NCT):
                    nc.tensor.matmul(
                        y_ps[:],
                        z_s[:, ct * TC + tt * 128: ct * TC + (tt + 1) * 128],
                        wcomb_s[:, ct * D + 512: ct * D + 1024],
                        start=(ct == 0), stop=(ct == NCT - 1))
                evict_tt(tt, 1, y_ps)

    nc.compile()
    return nc


@functools.lru_cache(maxsize=4)
def _program(flags):
    return _build_program(*flags)


def _v2m(v):
    return np.ascontiguousarray(np.asarray(v, np.float32).reshape(NCT, 128).T)


def kernel(x, W_ih_w, W_ih_b, W_gate_w, W_gate_b, W_decay_w, W_decay_b,
           W_z_w, W_z_b, conv_w, conv_b, ln_g, ln_b, W_out_w, W_out_b,
           out_w, out_b):
    f32 = np.float32
    bf = ml_dtypes.bfloat16
    x = np.asarray(x, f32)

    out_bias_eff = (np.asarray(W_out_b, f32) @ np.asarray(out_w, f32)
                    + np.asarray(out_b, f32))
    has_ob = bool(np.any(out_bias_eff != 0.0))
    has_lb = bool(np.any(np.asarray(ln_b) != 0.0))
    nc = _program((has_ob, has_lb))

    w4 = np.concatenate([np.asarray(W_ih_w, f32), np.asarray(W_gate_w, f32),
                         np.asarray(W_decay_w, f32), np.asarray(W_z_w, f32)],
                        axis=1).astype(bf)
    wcomb = (np.asarray(W_out_w, f32) @ np.asarray(out_w, f32)).astype(bf)
    convw_m = np.ascontiguousarray(
        np.asarray(conv_w, f32).reshape(NCT, 128, 4).transpose(1, 0, 2)
        .reshape(128, NCT * 4))
    bias_ih_m = _v2m(W_ih_b)
    shared = dict(
        w4=w4, wcomb=wcomb, bias_ih=bias_ih_m,
        bias_gneg=_v2m(-np.asarray(W_gate_b, f32)),
        bias_dec=_v2m(W_decay_b), bias_z=_v2m(W_z_b), conv_b=_v2m(conv_b),
        ln_g=_v2m(ln_g), convw=convw_m)
    if has_ob:
        shared["out_bias"] = np.ascontiguousarray(
            np.tile(out_bias_eff[None, :], (128, 1)).astype(f32))
    if has_lb:
        shared["ln_b"] = _v2m(ln_b)

    zero_halo = np.zeros((128, NCT), f32)
    in_maps = []
    for c in range(NCORES):
        b, k = divmod(c, KCHUNKS)
        t0 = k * TC
        if k == 0:
            xc = np.vstack([np.zeros((HALO, D), f32), x[b, :TC]])
        else:
            xc = x[b, t0 - HALO: t0 + TC]
        xTc = np.ascontiguousarray(xc.T).astype(bf)
        oh = np.zeros(NCORES, f32)
        if k > 0:
            oh[b * KCHUNKS + k - 1] = 1.0
        in_maps.append({
            **shared,
            "xT": xTc,
            "bias_halo": bias_ih_m if k > 0 else zero_halo,
            "onehot": np.ascontiguousarray(np.tile(oh[None, :], (128, 1))),
        })

    res = run_bass_kernel_spmd(nc, in_maps, core_ids=list(range(NCORES)))

    outf = np.empty((B, T, D), f32)
    for c in range(NCORES):
        b, k = divmod(c, KCHUNKS)
        outf[b, k * TC:(k + 1) * TC, :] = res.results[c]["out"]
    return outf
